# revision 2
# baseline (speedup 1.0000x reference)
"""DecodeBox (nms_detection) Trainium2 Bass kernel, 8-core data-parallel.

Reference computation (per element of [B=4, A=3, D=64, H=64, W=64]):
  out[b, n, 0] = (sigmoid(x0) + w) * 4        n = a*262144 + d*4096 + h*64 + w
  out[b, n, 1] = (sigmoid(x1) + h) * 4
  out[b, n, 2] = (sigmoid(x2) + d) * 4
  out[b, n, 3] = exp(x3) * anchor_w[a]        anchor_w = [10, 16, 33]
  out[b, n, 4:10] = sigmoid(x4..x9)
Input layout [B, 30, D, H, W] with channel = a*10 + attr; output [B, 786432, 10].

Strategy: the (b, a) slab pairs give 12 slabs of 262144 positions; split each
slab in half -> 24 half-slabs of 131072 positions, 3 per core (perfectly
balanced, pure data parallel).  Per half-slab, SBUF tiles are [128 partitions
x (attrs-major) free]; ACT computes tanh(x/2) (sigmoid = 0.5*tanh(x/2)+0.5,
same table set as exp -> zero table-set switches) and exp(x + ln(anchor))
writing directly with a stride-10 interleaved AP, so the output tile is in
final [pos, attr] order and the store DMA is fully contiguous.  DVE applies
the affine grid fixups ((sig+g)*4 == 2*t + (2+4g)) on the strided lanes.
All per-core-varying constants (grid-z offsets, anchor log-widths) are input
data, so one SPMD program serves all 8 cores.
"""

import numpy as np

B, A, ATTRS = 4, 3, 10
D = H = W = 64
S = D * H * W              # 262144 positions per (b, a) slab
SH = S // 2                # 131072 positions per half-slab
NCORES = 8
HS_PER_CORE = 3            # 24 half-slabs / 8 cores
P = 128                    # SBUF partitions
R = SH // P                # 1024 positions per partition per half-slab
F = 512                    # chunk of R per tile
NCHUNK = R // F            # 2
NT = HS_PER_CORE * NCHUNK  # 6 tiles per core
ANCHOR_W = np.array([10.0, 16.0, 33.0], dtype=np.float32)
NCONST = 2 * R + 2 * HS_PER_CORE

_CACHE = {}


def _build_nc():
    import concourse.bass as bass
    import concourse.mybir as mybir

    AFT = mybir.ActivationFunctionType
    add = mybir.AluOpType.add
    mult = mybir.AluOpType.mult
    f32 = mybir.dt.float32

    nc = bass.Bass()
    xin = nc.dram_tensor("xin", [HS_PER_CORE, ATTRS, SH], f32, kind="ExternalInput")
    consts = nc.dram_tensor("consts", [P, NCONST], f32, kind="ExternalInput")
    yout = nc.dram_tensor("yout", [HS_PER_CORE, SH, ATTRS], f32, kind="ExternalOutput")

    import contextlib

    with contextlib.ExitStack() as stack:
        ctile = stack.enter_context(nc.sbuf_tensor([P, NCONST], f32))
        in_t = [
            stack.enter_context(nc.sbuf_tensor(f"in{i}", [P, ATTRS * F], f32))
            for i in range(3)
        ]
        out_t = [
            stack.enter_context(nc.sbuf_tensor(f"out{i}", [P, ATTRS * F], f32))
            for i in range(3)
        ]
        const_done = stack.enter_context(nc.semaphore("const_done"))
        in_done = stack.enter_context(nc.semaphore("in_done"))
        out_done = stack.enter_context(nc.semaphore("out_done"))
        act_done = stack.enter_context(nc.semaphore("act_done"))
        dve_done = stack.enter_context(nc.semaphore("dve_done"))
        block = stack.enter_context(nc.Block())

        gx24 = ctile[:, 0:R]
        gy24 = ctile[:, R:2 * R]
        gzb = ctile[:, 2 * R:2 * R + HS_PER_CORE]
        lnanc = ctile[:, 2 * R + HS_PER_CORE:NCONST]

        def in_dram(i):
            hs, c = divmod(i, NCHUNK)
            return xin[hs].rearrange("a (p j) -> p a j", p=P)[:, :, c * F:(c + 1) * F]

        def out_dram(i):
            hs, c = divmod(i, NCHUNK)
            return yout[hs].rearrange("(p j) t -> p j t", p=P)[:, c * F:(c + 1) * F, :]

        @block.sync
        def _(sync):
            sync.dma_start(out=ctile[:, :], in_=consts[:, :]).then_inc(const_done, 16)
            pending_out = []

            def issue_out(k):
                sync.wait_ge(dve_done, k + 1)
                src = out_t[k % 3].rearrange("p (j t) -> p j t", t=ATTRS)
                sync.dma_start(out=out_dram(k), in_=src).then_inc(out_done, 16)

            for i in range(3):
                dst = in_t[i].rearrange("p (a j) -> p a j", a=ATTRS)
                sync.dma_start(out=dst, in_=in_dram(i)).then_inc(in_done, 16)
            for i in range(3, NT):
                issue_out(i - 3)
                sync.wait_ge(act_done, 3 * (i - 2))
                dst = in_t[i % 3].rearrange("p (a j) -> p a j", a=ATTRS)
                sync.dma_start(out=dst, in_=in_dram(i)).then_inc(in_done, 16)
            for k in range(NT - 3, NT):
                issue_out(k)

        @block.scalar
        def _(scalar):
            scalar.wait_ge(const_done, 16)
            for i in range(NT):
                hs = i // NCHUNK
                scalar.wait_ge(in_done, 16 * (i + 1))
                if i >= 3:
                    scalar.wait_ge(out_done, 16 * (i - 2))
                in_r = in_t[i % 3].rearrange("p (a j) -> p a j", a=ATTRS)
                out_r = out_t[i % 3].rearrange("p (j t) -> p t j", t=ATTRS)
                nc.scalar.activation(
                    out_r[:, 0:3, :], in_r[:, 0:3, :], AFT.Tanh, scale=0.5
                ).then_inc(act_done, 1)
                nc.scalar.activation(
                    out_r[:, 3:4, :], in_r[:, 3:4, :], AFT.Exp,
                    bias=lnanc[:, hs:hs + 1],
                ).then_inc(act_done, 1)
                nc.scalar.activation(
                    out_r[:, 4:10, :], in_r[:, 4:10, :], AFT.Tanh, scale=0.5
                ).then_inc(act_done, 1)

        @block.vector
        def _(vector):
            vector.wait_ge(const_done, 16)
            for i in range(NT):
                hs, c = divmod(i, NCHUNK)
                out_r = out_t[i % 3].rearrange("p (j t) -> p t j", t=ATTRS)
                vector.wait_ge(act_done, 3 * i + 1)
                jsl = slice(c * F, (c + 1) * F)
                v0 = out_r[:, 0, :]
                nc.vector.tensor_scalar_mul(v0, v0, 2.0)
                nc.vector.tensor_tensor(v0, v0, gx24[:, jsl], add)
                v1 = out_r[:, 1, :]
                nc.vector.tensor_scalar_mul(v1, v1, 2.0)
                nc.vector.tensor_tensor(v1, v1, gy24[:, jsl], add)
                v2 = out_r[:, 2, :]
                nc.vector.tensor_scalar(v2, v2, 2.0, gzb[:, hs:hs + 1], mult, add)
                vector.wait_ge(act_done, 3 * i + 3)
                v49 = out_r[:, 4:10, :]
                nc.vector.tensor_scalar(v49, v49, 0.5, 0.5, mult, add).then_inc(dve_done, 1)

    return nc


def _host_constants():
    """[P, NCONST] consts per core: gx24 | gy24 | gzb | lnanc.

    Position s within a half-slab decomposes as s = p*R + jj (jj in [0,R)):
      w = jj % 64;  hgrid = 16*(p%4) + jj//64;  d = half*32 + p//4
    Lanes 0-2 compute 2*tanh(x/2) + (2 + 4*grid) == (sigmoid(x)+grid)*4.
    """
    p = np.arange(P)
    jj = np.arange(R)
    gx24 = np.broadcast_to(2.0 + 4.0 * (jj % W), (P, R))
    gy24 = 2.0 + 4.0 * (16.0 * (p[:, None] % 4) + jj[None, :] // W)
    out = []
    for core in range(NCORES):
        gzb = np.empty((P, HS_PER_CORE), np.float32)
        lnanc = np.empty((P, HS_PER_CORE), np.float32)
        for k in range(HS_PER_CORE):
            hs_g = HS_PER_CORE * core + k
            slab, half = divmod(hs_g, 2)
            a = slab % A
            gzb[:, k] = 2.0 + 4.0 * (32.0 * half + p // 4)
            lnanc[:, k] = np.log(ANCHOR_W[a])
        out.append(
            np.concatenate([gx24, gy24, gzb, lnanc], axis=1).astype(np.float32)
        )
    return out


def _run(inputs, trace=False):
    from concourse.bass_utils import run_bass_kernel_spmd

    x = np.ascontiguousarray(np.asarray(inputs["input"], dtype=np.float32))
    assert x.shape == (B, A * ATTRS, D, H, W), x.shape
    x12 = x.reshape(B * A, ATTRS, S)

    if "nc" not in _CACHE:
        _CACHE["nc"] = _build_nc()
        _CACHE["consts"] = _host_constants()
    nc = _CACHE["nc"]
    consts = _CACHE["consts"]

    in_maps = []
    for core in range(NCORES):
        parts = []
        for k in range(HS_PER_CORE):
            hs_g = HS_PER_CORE * core + k
            slab, half = divmod(hs_g, 2)
            parts.append(x12[slab, :, half * SH:(half + 1) * SH])
        in_maps.append({"xin": np.stack(parts), "consts": consts[core]})

    res = run_bass_kernel_spmd(
        nc, in_maps, core_ids=list(range(NCORES)), trace=trace
    )
    _CACHE["last_exec_ns"] = res.exec_time_ns
    _CACHE["last_results"] = res

    full = np.stack([res.results[c]["yout"] for c in range(NCORES)])
    return full.reshape(B, A * S, ATTRS)


def kernel(**inputs):
    return _run(inputs, trace=False)


# revision 3
# speedup vs baseline: 1.0224x; 1.0224x over previous
"""DecodeBox (nms_detection) Trainium2 Bass kernel, 8-core data-parallel.

Reference computation (per element of [B=4, A=3, D=64, H=64, W=64]):
  out[b, n, 0] = (sigmoid(x0) + w) * 4        n = a*262144 + d*4096 + h*64 + w
  out[b, n, 1] = (sigmoid(x1) + h) * 4
  out[b, n, 2] = (sigmoid(x2) + d) * 4
  out[b, n, 3] = exp(x3) * anchor_w[a]        anchor_w = [10, 16, 33]
  out[b, n, 4:10] = sigmoid(x4..x9)
Input layout [B, 30, D, H, W] with channel = a*10 + attr; output [B, 786432, 10].

Strategy: the (b, a) pairs give 12 slabs of 262144 positions; split each slab
in half -> 24 half-slabs of 131072 positions, 3 per core (perfectly balanced,
pure data parallel; host only slices/stacks along existing axes).  Per
half-slab, SBUF tiles are [128 partitions x (attrs-major) free]; ACT computes
tanh(x/2) (sigmoid = 0.5*tanh(x/2)+0.5 -- tanh and exp share one activation
table set, so zero table-set switches) and exp(x + ln(anchor)) writing
directly through a stride-10 interleaved AP, so the output tile is in final
[pos, attr] order and the store DMA is fully contiguous.  DVE applies the
affine grid fixups ((sig+g)*4 == 2*t + (2+4g)) on the strided lanes, reading
tiny broadcast tables (stride-0 APs) instead of full-size grids.  All
per-core-varying constants are input data, so one SPMD program serves all 8
cores.  The kernel is HBM-bound: ~31.5 MB/core of unavoidable traffic at
~358 GB/s per-NC => ~88 us floor.
"""

import numpy as np

B, A, ATTRS = 4, 3, 10
D = H = W = 64
S = D * H * W              # 262144 positions per (b, a) slab
SH = S // 2                # 131072 positions per half-slab
NCORES = 8
HS_PER_CORE = 3            # 24 half-slabs / 8 cores
P = 128                    # SBUF partitions
R = SH // P                # 1024 positions per partition per half-slab
F = 512                    # chunk of R per tile
F1 = F // W                # 8 coarse rows per chunk
NCHUNK = R // F            # 2
NT = HS_PER_CORE * NCHUNK  # 6 tiles per core
ANCHOR_W = np.array([10.0, 16.0, 33.0], dtype=np.float32)
# const layout (columns of [P, NCONST]): gxrow | stair | pv | gzb | lnanc
NCONST = W + F1 + NCHUNK + HS_PER_CORE + HS_PER_CORE

_CACHE = {}


def _build_nc():
    import contextlib

    import concourse.bass as bass
    import concourse.mybir as mybir

    AFT = mybir.ActivationFunctionType
    add = mybir.AluOpType.add
    mult = mybir.AluOpType.mult
    f32 = mybir.dt.float32

    nc = bass.Bass()
    xin = nc.dram_tensor("xin", [HS_PER_CORE, ATTRS, SH], f32, kind="ExternalInput")
    consts = nc.dram_tensor("consts", [P, NCONST], f32, kind="ExternalInput")
    yout = nc.dram_tensor("yout", [HS_PER_CORE, SH, ATTRS], f32, kind="ExternalOutput")

    with contextlib.ExitStack() as stack:
        ctile = stack.enter_context(nc.sbuf_tensor("ctile", [P, NCONST], f32))
        in_t = [
            stack.enter_context(nc.sbuf_tensor(f"in{i}", [P, ATTRS * F], f32))
            for i in range(3)
        ]
        out_t = [
            stack.enter_context(nc.sbuf_tensor(f"out{i}", [P, ATTRS * F], f32))
            for i in range(3)
        ]
        const_done = stack.enter_context(nc.semaphore("const_done"))
        in_done = stack.enter_context(nc.semaphore("in_done"))
        out_done = stack.enter_context(nc.semaphore("out_done"))
        act_done = stack.enter_context(nc.semaphore("act_done"))
        dve_done = stack.enter_context(nc.semaphore("dve_done"))
        block = stack.enter_context(nc.Block())

        o = 0
        gxrow = ctile[:, o:o + W]; o += W          # 2 + 4*w          [P, 64]
        stair = ctile[:, o:o + F1]; o += F1        # 4*j1             [P, 8]
        pv = ctile[:, o:o + NCHUNK]; o += NCHUNK   # 2+32c+64*(p%4)   [P, 2]
        gzb = ctile[:, o:o + HS_PER_CORE]; o += HS_PER_CORE   # (sig+d)*4 bias
        lnanc = ctile[:, o:o + HS_PER_CORE]        # ln(anchor_w[a])  [P, 3]

        def in_dram(i):
            hs, c = divmod(i, NCHUNK)
            return xin[hs].rearrange("a (p j) -> p a j", p=P)[:, :, c * F:(c + 1) * F]

        def out_dram(i):
            hs, c = divmod(i, NCHUNK)
            return yout[hs].rearrange("(p j) t -> p j t", p=P)[:, c * F:(c + 1) * F, :]

        @block.gpsimd
        def _(gpsimd):
            # tiny (40 KB) const load on the SWDGE ring so the HWDGE ring
            # streams the first input tile from t=0.
            gpsimd.dma_start(out=ctile[:, :], in_=consts[:, :]).then_inc(const_done, 16)

        @block.sync
        def _(sync):
            def issue_out(k):
                sync.wait_ge(dve_done, k + 1)
                src = out_t[k % 3].rearrange("p (j t) -> p j t", t=ATTRS)
                sync.dma_start(out=out_dram(k), in_=src).then_inc(out_done, 16)

            for i in range(3):
                dst = in_t[i].rearrange("p (a j) -> p a j", a=ATTRS)
                sync.dma_start(out=dst, in_=in_dram(i)).then_inc(in_done, 16)
            for i in range(3, NT):
                issue_out(i - 3)
                sync.wait_ge(act_done, 3 * (i - 2))
                dst = in_t[i % 3].rearrange("p (a j) -> p a j", a=ATTRS)
                sync.dma_start(out=dst, in_=in_dram(i)).then_inc(in_done, 16)
            for k in range(NT - 3, NT):
                issue_out(k)

        @block.scalar
        def _(scalar):
            for i in range(NT):
                hs = i // NCHUNK
                scalar.wait_ge(in_done, 16 * (i + 1))
                if i == 0:
                    scalar.wait_ge(const_done, 16)  # lnanc for A2
                if i >= 3:
                    scalar.wait_ge(out_done, 16 * (i - 2))
                in_r = in_t[i % 3].rearrange("p (a j) -> p a j", a=ATTRS)
                out_r = out_t[i % 3].rearrange("p (j t) -> p t j", t=ATTRS)
                nc.scalar.activation(
                    out_r[:, 0:3, :], in_r[:, 0:3, :], AFT.Tanh, scale=0.5
                ).then_inc(act_done, 1)
                nc.scalar.activation(
                    out_r[:, 3:4, :], in_r[:, 3:4, :], AFT.Exp,
                    bias=lnanc[:, hs:hs + 1],
                ).then_inc(act_done, 1)
                nc.scalar.activation(
                    out_r[:, 4:10, :], in_r[:, 4:10, :], AFT.Tanh, scale=0.5
                ).then_inc(act_done, 1)

        @block.vector
        def _(vector):
            vector.wait_ge(const_done, 16)
            gx_bc = gxrow.unsqueeze(1).broadcast_to([P, F1, W])
            st_bc = stair.unsqueeze(2).broadcast_to([P, F1, W])
            for i in range(NT):
                hs, c = divmod(i, NCHUNK)
                out_r = out_t[i % 3].rearrange("p (j t) -> p t j", t=ATTRS)
                out_r4 = out_t[i % 3].rearrange(
                    "p (j1 j0 t) -> p t j1 j0", t=ATTRS, j0=W
                )
                vector.wait_ge(act_done, 3 * i + 1)
                v0, v0f = out_r4[:, 0, :, :], out_r[:, 0, :]
                nc.vector.tensor_scalar_mul(v0f, v0f, 2.0)
                nc.vector.tensor_tensor(v0, v0, gx_bc, add)
                v1, v1f = out_r4[:, 1, :, :], out_r[:, 1, :]
                nc.vector.tensor_scalar(v1f, v1f, 2.0, pv[:, c:c + 1], mult, add)
                nc.vector.tensor_tensor(v1, v1, st_bc, add)
                v2f = out_r[:, 2, :]
                nc.vector.tensor_scalar(v2f, v2f, 2.0, gzb[:, hs:hs + 1], mult, add)
                vector.wait_ge(act_done, 3 * i + 3)
                v49 = out_r[:, 4:10, :]
                nc.vector.tensor_scalar(
                    v49, v49, 0.5, 0.5, mult, add
                ).then_inc(dve_done, 1)

    return nc


def _host_constants():
    """[P, NCONST] per core: gxrow | stair | pv | gzb | lnanc.

    Half-slab position s = p*R + jj, jj = c*F + j1*64 + j0:
      w = j0;  hgrid = 16*(p%4) + c*8 + j1;  d = half*32 + p//4
    Lanes 0-2 hold t = tanh(x/2); output = 2*t + (2 + 4*grid).
    """
    p = np.arange(P)
    cols = []
    cols.append(np.broadcast_to(2.0 + 4.0 * np.arange(W), (P, W)))          # gxrow
    cols.append(np.broadcast_to(4.0 * np.arange(F1), (P, F1)))              # stair
    cols.append(2.0 + 32.0 * np.arange(NCHUNK)[None, :] + 64.0 * (p[:, None] % 4))
    base = np.concatenate(cols, axis=1)
    out = []
    for core in range(NCORES):
        gzb = np.empty((P, HS_PER_CORE), np.float32)
        lnanc = np.empty((P, HS_PER_CORE), np.float32)
        for k in range(HS_PER_CORE):
            hs_g = HS_PER_CORE * core + k
            slab, half = divmod(hs_g, 2)
            gzb[:, k] = 2.0 + 128.0 * half + 4.0 * (p // 4)
            lnanc[:, k] = np.log(ANCHOR_W[slab % A])
        out.append(np.concatenate([base, gzb, lnanc], axis=1).astype(np.float32))
    return out


def _run(inputs, trace=False):
    from concourse.bass_utils import run_bass_kernel_spmd

    x = np.ascontiguousarray(np.asarray(inputs["input"], dtype=np.float32))
    assert x.shape == (B, A * ATTRS, D, H, W), x.shape
    x12 = x.reshape(B * A, ATTRS, S)

    if "nc" not in _CACHE:
        _CACHE["nc"] = _build_nc()
        _CACHE["consts"] = _host_constants()
    nc = _CACHE["nc"]
    consts = _CACHE["consts"]

    in_maps = []
    for core in range(NCORES):
        parts = []
        for k in range(HS_PER_CORE):
            hs_g = HS_PER_CORE * core + k
            slab, half = divmod(hs_g, 2)
            parts.append(x12[slab, :, half * SH:(half + 1) * SH])
        in_maps.append({"xin": np.stack(parts), "consts": consts[core]})

    res = run_bass_kernel_spmd(
        nc, in_maps, core_ids=list(range(NCORES)), trace=trace
    )
    _CACHE["last_exec_ns"] = res.exec_time_ns
    _CACHE["last_results"] = res

    full = np.stack([res.results[c]["yout"] for c in range(NCORES)])
    return full.reshape(B, A * S, ATTRS)


def kernel(**inputs):
    return _run(inputs, trace=False)


# revision 4
# speedup vs baseline: 1.0342x; 1.0116x over previous
"""DecodeBox (nms_detection) Trainium2 Bass kernel, 8-core data-parallel.

Reference computation (per element of [B=4, A=3, D=64, H=64, W=64]):
  out[b, n, 0] = (sigmoid(x0) + w) * 4        n = a*262144 + d*4096 + h*64 + w
  out[b, n, 1] = (sigmoid(x1) + h) * 4
  out[b, n, 2] = (sigmoid(x2) + d) * 4
  out[b, n, 3] = exp(x3) * anchor_w[a]        anchor_w = [10, 16, 33]
  out[b, n, 4:10] = sigmoid(x4..x9)
Input layout [B, 30, D, H, W] with channel = a*10 + attr; output [B, 786432, 10].

Strategy: the (b, a) pairs give 12 slabs of 262144 positions; split each slab
in half -> 24 half-slabs of 131072 positions, 3 per core (perfectly balanced,
pure data parallel; host only slices/stacks along existing axes).

Per half-slab chunk, SBUF tiles are [128 partitions x free].  The kernel is
HBM-bound (~31.5 MB/core unavoidable traffic, ~385 GB/s streaming), so both
compute engines just have to keep up with the DMA ring:
  - ACT computes tanh(x/2) IN-PLACE on the (contiguous, attr-major) input
    tile at 1 elem/cycle (sigmoid = 0.5*tanh(x/2)+0.5; tanh and exp share one
    activation table set, so zero table-set switches), plus exp(x+ln(anchor))
    written straight to the interleaved output lane (final value).
  - DVE reads the contiguous tanh lanes and writes the final values through
    stride-10 interleaved APs (strided writes cost ~2x on ACT but much less
    on DVE), fusing the affine grid adds: (sig+g)*4 == 2*t + (2+4g), one
    scalar_tensor_tensor / tensor_scalar op per lane group, with the grid
    tables read through tiny stride-0 broadcast APs.
The output tile ends up in final [pos, attr] order so the store DMA is fully
contiguous.  All per-core-varying constants are input data, so one SPMD
program serves all 8 cores.
"""

import numpy as np

B, A, ATTRS = 4, 3, 10
D = H = W = 64
S = D * H * W              # 262144 positions per (b, a) slab
SH = S // 2                # 131072 positions per half-slab
NCORES = 8
HS_PER_CORE = 3            # 24 half-slabs / 8 cores
P = 128                    # SBUF partitions
R = SH // P                # 1024 positions per partition per half-slab
F = 512                    # chunk of R per tile
F1 = F // W                # 8 coarse rows per chunk
NCHUNK = R // F            # 2
NT = HS_PER_CORE * NCHUNK  # 6 tiles per core
ANCHOR_W = np.array([10.0, 16.0, 33.0], dtype=np.float32)
# const layout (columns of [P, NCONST]): gxrow(64) | gysm(16) | gzb(3) | lnanc(3)
NCONST = W + NCHUNK * F1 + HS_PER_CORE + HS_PER_CORE

_CACHE = {}


def _build_nc():
    import contextlib

    import concourse.bass as bass
    import concourse.mybir as mybir

    AFT = mybir.ActivationFunctionType
    add = mybir.AluOpType.add
    mult = mybir.AluOpType.mult
    f32 = mybir.dt.float32

    nc = bass.Bass()
    xin = nc.dram_tensor("xin", [HS_PER_CORE, ATTRS, SH], f32, kind="ExternalInput")
    consts = nc.dram_tensor("consts", [P, NCONST], f32, kind="ExternalInput")
    yout = nc.dram_tensor("yout", [HS_PER_CORE, SH, ATTRS], f32, kind="ExternalOutput")

    with contextlib.ExitStack() as stack:
        ctile = stack.enter_context(nc.sbuf_tensor("ctile", [P, NCONST], f32))
        in_t = [
            stack.enter_context(nc.sbuf_tensor(f"in{i}", [P, ATTRS * F], f32))
            for i in range(3)
        ]
        out_t = [
            stack.enter_context(nc.sbuf_tensor(f"out{i}", [P, ATTRS * F], f32))
            for i in range(3)
        ]
        const_done = stack.enter_context(nc.semaphore("const_done"))
        in_done = stack.enter_context(nc.semaphore("in_done"))
        out_done = stack.enter_context(nc.semaphore("out_done"))
        act_done = stack.enter_context(nc.semaphore("act_done"))
        dve_done = stack.enter_context(nc.semaphore("dve_done"))
        block = stack.enter_context(nc.Block())

        o = 0
        gxrow = ctile[:, o:o + W]; o += W                     # 2 + 4*j0   [P, 64]
        gysm = ctile[:, o:o + NCHUNK * F1]; o += NCHUNK * F1  # [P, 16]
        gzb = ctile[:, o:o + HS_PER_CORE]; o += HS_PER_CORE   # z-lane bias
        lnanc = ctile[:, o:o + HS_PER_CORE]                   # ln(anchor_w[a])

        def in_dram(i):
            hs, c = divmod(i, NCHUNK)
            return xin[hs].rearrange("a (p j) -> p a j", p=P)[:, :, c * F:(c + 1) * F]

        def out_dram(i):
            hs, c = divmod(i, NCHUNK)
            return yout[hs].rearrange("(p j) t -> p j t", p=P)[:, c * F:(c + 1) * F, :]

        @block.gpsimd
        def _(gpsimd):
            # tiny const load on the SWDGE ring so the HWDGE ring streams the
            # first input tile from t=0.
            gpsimd.dma_start(out=ctile[:, :], in_=consts[:, :]).then_inc(const_done, 16)

        @block.sync
        def _(sync):
            def issue_out(k):
                sync.wait_ge(dve_done, k + 1)
                src = out_t[k % 3].rearrange("p (j t) -> p j t", t=ATTRS)
                sync.dma_start(out=out_dram(k), in_=src).then_inc(out_done, 16)

            for i in range(3):
                dst = in_t[i].rearrange("p (a j) -> p a j", a=ATTRS)
                sync.dma_start(out=dst, in_=in_dram(i)).then_inc(in_done, 16)
            for i in range(3, NT):
                issue_out(i - 3)
                sync.wait_ge(act_done, 3 * (i - 2))
                dst = in_t[i % 3].rearrange("p (a j) -> p a j", a=ATTRS)
                sync.dma_start(out=dst, in_=in_dram(i)).then_inc(in_done, 16)
            for k in range(NT - 3, NT):
                issue_out(k)

        @block.scalar
        def _(scalar):
            for i in range(NT):
                hs = i // NCHUNK
                scalar.wait_ge(in_done, 16 * (i + 1))
                if i == 0:
                    scalar.wait_ge(const_done, 16)  # lnanc for A2
                if i >= 3:
                    scalar.wait_ge(out_done, 16 * (i - 2))
                in_r = in_t[i % 3].rearrange("p (a j) -> p a j", a=ATTRS)
                out_r = out_t[i % 3].rearrange("p (j t) -> p t j", t=ATTRS)
                # tanh in-place (contiguous: 1 elem/cycle); exp straight to the
                # interleaved lane (final value, no DVE fixup needed).
                nc.scalar.activation(
                    in_r[:, 0:3, :], in_r[:, 0:3, :], AFT.Tanh, scale=0.5
                ).then_inc(act_done, 1)
                nc.scalar.activation(
                    out_r[:, 3:4, :], in_r[:, 3:4, :], AFT.Exp,
                    bias=lnanc[:, hs:hs + 1],
                ).then_inc(act_done, 1)
                nc.scalar.activation(
                    in_r[:, 4:10, :], in_r[:, 4:10, :], AFT.Tanh, scale=0.5
                ).then_inc(act_done, 1)

        @block.vector
        def _(vector):
            vector.wait_ge(const_done, 16)
            gx_bc = gxrow.unsqueeze(1).broadcast_to([P, F1, W])
            for i in range(NT):
                hs, c = divmod(i, NCHUNK)
                in_r = in_t[i % 3].rearrange("p (a j) -> p a j", a=ATTRS)
                in_r4 = in_t[i % 3].rearrange(
                    "p (a j1 j0) -> p a j1 j0", a=ATTRS, j0=W
                )
                out_r = out_t[i % 3].rearrange("p (j t) -> p t j", t=ATTRS)
                out_r4 = out_t[i % 3].rearrange(
                    "p (j1 j0 t) -> p t j1 j0", t=ATTRS, j0=W
                )
                gy_bc = gysm[:, c * F1:(c + 1) * F1].unsqueeze(2).broadcast_to(
                    [P, F1, W]
                )
                vector.wait_ge(act_done, 3 * i + 1)
                nc.vector.scalar_tensor_tensor(
                    out_r4[:, 0], in_r4[:, 0], 2.0, gx_bc, mult, add
                )
                nc.vector.scalar_tensor_tensor(
                    out_r4[:, 1], in_r4[:, 1], 2.0, gy_bc, mult, add
                )
                nc.vector.tensor_scalar(
                    out_r[:, 2, :], in_r[:, 2, :], 2.0, gzb[:, hs:hs + 1], mult, add
                )
                vector.wait_ge(act_done, 3 * i + 3)
                nc.vector.tensor_scalar(
                    out_r[:, 4:10, :], in_r[:, 4:10, :], 0.5, 0.5, mult, add
                ).then_inc(dve_done, 1)

    return nc


def _host_constants():
    """[P, NCONST] per core: gxrow | gysm | gzb | lnanc.

    Half-slab position s = p*R + jj, jj = c*F + j1*64 + j0:
      w = j0;  hgrid = 16*(p%4) + c*8 + j1;  d = half*32 + p//4
    Lanes 0-2 hold t = tanh(x/2); output = 2*t + (2 + 4*grid).
    """
    p = np.arange(P)
    gxrow = np.broadcast_to(2.0 + 4.0 * np.arange(W), (P, W))
    cj = np.arange(NCHUNK * F1)  # c*8 + j1
    gysm = 2.0 + 4.0 * (16.0 * (p[:, None] % 4) + cj[None, :])
    base = np.concatenate([gxrow, gysm], axis=1)
    out = []
    for core in range(NCORES):
        gzb = np.empty((P, HS_PER_CORE), np.float32)
        lnanc = np.empty((P, HS_PER_CORE), np.float32)
        for k in range(HS_PER_CORE):
            hs_g = HS_PER_CORE * core + k
            slab, half = divmod(hs_g, 2)
            gzb[:, k] = 2.0 + 128.0 * half + 4.0 * (p // 4)
            lnanc[:, k] = np.log(ANCHOR_W[slab % A])
        out.append(np.concatenate([base, gzb, lnanc], axis=1).astype(np.float32))
    return out


def _run(inputs, trace=False):
    from concourse.bass_utils import run_bass_kernel_spmd

    x = np.ascontiguousarray(np.asarray(inputs["input"], dtype=np.float32))
    assert x.shape == (B, A * ATTRS, D, H, W), x.shape
    x12 = x.reshape(B * A, ATTRS, S)

    if "nc" not in _CACHE:
        _CACHE["nc"] = _build_nc()
        _CACHE["consts"] = _host_constants()
    nc = _CACHE["nc"]
    consts = _CACHE["consts"]

    in_maps = []
    for core in range(NCORES):
        parts = []
        for k in range(HS_PER_CORE):
            hs_g = HS_PER_CORE * core + k
            slab, half = divmod(hs_g, 2)
            parts.append(x12[slab, :, half * SH:(half + 1) * SH])
        in_maps.append({"xin": np.stack(parts), "consts": consts[core]})

    res = run_bass_kernel_spmd(
        nc, in_maps, core_ids=list(range(NCORES)), trace=trace
    )
    _CACHE["last_exec_ns"] = res.exec_time_ns
    _CACHE["last_results"] = res

    full = np.stack([res.results[c]["yout"] for c in range(NCORES)])
    return full.reshape(B, A * S, ATTRS)


def kernel(**inputs):
    return _run(inputs, trace=False)


# revision 6
# speedup vs baseline: 1.1268x; 1.0895x over previous
"""DecodeBox (nms_detection) Trainium2 Bass kernel, 8-core data-parallel.

Reference computation (per element of [B=4, A=3, D=64, H=64, W=64]):
  out[b, n, 0] = (sigmoid(x0) + w) * 4        n = a*262144 + d*4096 + h*64 + w
  out[b, n, 1] = (sigmoid(x1) + h) * 4
  out[b, n, 2] = (sigmoid(x2) + d) * 4
  out[b, n, 3] = exp(x3) * anchor_w[a]        anchor_w = [10, 16, 33]
  out[b, n, 4:10] = sigmoid(x4..x9)
Input layout [B, 30, D, H, W] with channel = a*10 + attr; output [B, 786432, 10].

Strategy: the (b, a) pairs give 12 slabs of 262144 positions; split each slab
in half -> 24 half-slabs of 131072 positions, 3 per core (perfectly balanced,
pure data parallel; host only slices/stacks along existing axes).

Per half-slab chunk, SBUF tiles are [128 partitions x free].  The kernel is
HBM-bound (~31.5 MB/core unavoidable traffic, ~385 GB/s streaming), so both
compute engines just have to keep up with the DMA ring:
  - ACT computes tanh(x/2) IN-PLACE on the (contiguous, attr-major) input
    tile at 1 elem/cycle (sigmoid = 0.5*tanh(x/2)+0.5; tanh and exp share one
    activation table set, so zero table-set switches), plus exp(x+ln(anchor))
    written straight to the interleaved output lane (final value).
  - DVE reads the contiguous tanh lanes and writes the final values through
    stride-10 interleaved APs (strided writes cost ~2x on ACT but much less
    on DVE), fusing the affine grid adds: (sig+g)*4 == 2*t + (2+4g), one
    scalar_tensor_tensor / tensor_scalar op per lane group, with the grid
    tables read through tiny stride-0 broadcast APs.
The output tile ends up in final [pos, attr] order so the store DMA is fully
contiguous.  All per-core-varying constants are input data, so one SPMD
program serves all 8 cores.
"""

import numpy as np

B, A, ATTRS = 4, 3, 10
D = H = W = 64
S = D * H * W              # 262144 positions per (b, a) slab
SH = S // 2                # 131072 positions per half-slab
NCORES = 8
HS_PER_CORE = 3            # 24 half-slabs / 8 cores
P = 128                    # SBUF partitions
R = SH // P                # 1024 positions per partition per half-slab
F = 512                    # chunk of R per tile
F1 = F // W                # 8 coarse rows per chunk
NCHUNK = R // F            # 2
NT = HS_PER_CORE * NCHUNK  # 6 tiles per core
ANCHOR_W = np.array([10.0, 16.0, 33.0], dtype=np.float32)
# const layout (columns of [P, NCONST]): gxrow(64) | gysm(16) | gzb(3) | lnanc(3)
NCONST = W + NCHUNK * F1 + HS_PER_CORE + HS_PER_CORE

_CACHE = {}


def _build_nc():
    import contextlib

    import concourse.bass as bass
    import concourse.mybir as mybir

    AFT = mybir.ActivationFunctionType
    add = mybir.AluOpType.add
    mult = mybir.AluOpType.mult
    f32 = mybir.dt.float32

    nc = bass.Bass()
    xin = nc.dram_tensor("xin", [HS_PER_CORE, ATTRS, SH], f32, kind="ExternalInput")
    consts = nc.dram_tensor("consts", [P, NCONST], f32, kind="ExternalInput")
    yout = nc.dram_tensor("yout", [HS_PER_CORE, SH, ATTRS], f32, kind="ExternalOutput")

    with contextlib.ExitStack() as stack:
        ctile = stack.enter_context(nc.sbuf_tensor("ctile", [P, NCONST], f32))
        in_t = [
            stack.enter_context(nc.sbuf_tensor(f"in{i}", [P, ATTRS * F], f32))
            for i in range(3)
        ]
        out_t = [
            stack.enter_context(nc.sbuf_tensor(f"out{i}", [P, ATTRS * F], f32))
            for i in range(3)
        ]
        const_done = stack.enter_context(nc.semaphore("const_done"))
        in_done = stack.enter_context(nc.semaphore("in_done"))
        out_done = stack.enter_context(nc.semaphore("out_done"))
        act_done = stack.enter_context(nc.semaphore("act_done"))
        dve_done = stack.enter_context(nc.semaphore("dve_done"))
        block = stack.enter_context(nc.Block())

        o = 0
        gxrow = ctile[:, o:o + W]; o += W                     # 2 + 4*j0   [P, 64]
        gysm = ctile[:, o:o + NCHUNK * F1]; o += NCHUNK * F1  # [P, 16]
        gzb = ctile[:, o:o + HS_PER_CORE]; o += HS_PER_CORE   # z-lane bias
        lnanc = ctile[:, o:o + HS_PER_CORE]                   # ln(anchor_w[a])

        def in_dram(i):
            hs, c = divmod(i, NCHUNK)
            return xin[hs].rearrange("a (p j) -> p a j", p=P)[:, :, c * F:(c + 1) * F]

        def out_dram(i):
            hs, c = divmod(i, NCHUNK)
            return yout[hs].rearrange("(p j) t -> p j t", p=P)[:, c * F:(c + 1) * F, :]

        @block.gpsimd
        def _(gpsimd):
            # tiny const load on the SWDGE ring so the HWDGE ring streams the
            # first input tile from t=0.
            gpsimd.dma_start(out=ctile[:, :], in_=consts[:, :]).then_inc(const_done, 16)

        @block.sync
        def _(sync):
            def issue_out(k):
                sync.wait_ge(dve_done, k + 1)
                src = out_t[k % 3].rearrange("p (j t) -> p j t", t=ATTRS)
                sync.dma_start(out=out_dram(k), in_=src).then_inc(out_done, 16)

            for i in range(3):
                dst = in_t[i].rearrange("p (a j) -> p a j", a=ATTRS)
                sync.dma_start(out=dst, in_=in_dram(i)).then_inc(in_done, 16)
            for i in range(3, NT):
                issue_out(i - 3)
                sync.wait_ge(act_done, 3 * (i - 2))
                dst = in_t[i % 3].rearrange("p (a j) -> p a j", a=ATTRS)
                sync.dma_start(out=dst, in_=in_dram(i)).then_inc(in_done, 16)
            for k in range(NT - 3, NT):
                issue_out(k)

        @block.scalar
        def _(scalar):
            for i in range(NT):
                hs = i // NCHUNK
                scalar.wait_ge(in_done, 16 * (i + 1))
                if i == 0:
                    scalar.wait_ge(const_done, 16)  # lnanc for A2
                if i >= 3:
                    scalar.wait_ge(out_done, 16 * (i - 2))
                in_r = in_t[i % 3].rearrange("p (a j) -> p a j", a=ATTRS)
                out_r = out_t[i % 3].rearrange("p (j t) -> p t j", t=ATTRS)
                # tanh in-place (contiguous: 1 elem/cycle); exp straight to the
                # interleaved lane (final value, no DVE fixup needed).
                nc.scalar.activation(
                    in_r[:, 0:3, :], in_r[:, 0:3, :], AFT.Tanh, scale=0.5
                ).then_inc(act_done, 1)
                nc.scalar.activation(
                    out_r[:, 3:4, :], in_r[:, 3:4, :], AFT.Exp,
                    bias=lnanc[:, hs:hs + 1],
                ).then_inc(act_done, 1)
                nc.scalar.activation(
                    in_r[:, 4:10, :], in_r[:, 4:10, :], AFT.Tanh, scale=0.5
                ).then_inc(act_done, 1)

        @block.vector
        def _(vector):
            vector.wait_ge(const_done, 16)
            gx_bc = gxrow.unsqueeze(1).broadcast_to([P, F1, W])
            for i in range(NT):
                hs, c = divmod(i, NCHUNK)
                in_r = in_t[i % 3].rearrange("p (a j) -> p a j", a=ATTRS)
                in_r4 = in_t[i % 3].rearrange(
                    "p (a j1 j0) -> p a j1 j0", a=ATTRS, j0=W
                )
                out_r = out_t[i % 3].rearrange("p (j t) -> p t j", t=ATTRS)
                out_r4 = out_t[i % 3].rearrange(
                    "p (j1 j0 t) -> p t j1 j0", t=ATTRS, j0=W
                )
                gy_bc = gysm[:, c * F1:(c + 1) * F1].unsqueeze(2).broadcast_to(
                    [P, F1, W]
                )
                vector.wait_ge(act_done, 3 * i + 1)
                nc.vector.scalar_tensor_tensor(
                    out_r4[:, 0], in_r4[:, 0], 2.0, gx_bc, mult, add
                )
                nc.vector.scalar_tensor_tensor(
                    out_r4[:, 1], in_r4[:, 1], 2.0, gy_bc, mult, add
                )
                nc.vector.tensor_scalar(
                    out_r[:, 2, :], in_r[:, 2, :], 2.0, gzb[:, hs:hs + 1], mult, add
                )
                vector.wait_ge(act_done, 3 * i + 3)
                nc.vector.tensor_scalar(
                    out_r[:, 4:10, :], in_r[:, 4:10, :], 0.5, 0.5, mult, add
                ).then_inc(dve_done, 1)

    return nc


def _host_constants():
    """[P, NCONST] per core: gxrow | gysm | gzb | lnanc.

    Half-slab position s = p*R + jj, jj = c*F + j1*64 + j0:
      w = j0;  hgrid = 16*(p%4) + c*8 + j1;  d = half*32 + p//4
    Lanes 0-2 hold t = tanh(x/2); output = 2*t + (2 + 4*grid).
    """
    p = np.arange(P)
    gxrow = np.broadcast_to(2.0 + 4.0 * np.arange(W), (P, W))
    cj = np.arange(NCHUNK * F1)  # c*8 + j1
    gysm = 2.0 + 4.0 * (16.0 * (p[:, None] % 4) + cj[None, :])
    base = np.concatenate([gxrow, gysm], axis=1)
    out = []
    for core in range(NCORES):
        gzb = np.empty((P, HS_PER_CORE), np.float32)
        lnanc = np.empty((P, HS_PER_CORE), np.float32)
        for k in range(HS_PER_CORE):
            hs_g = HS_PER_CORE * core + k
            slab, half = divmod(hs_g, 2)
            gzb[:, k] = 2.0 + 128.0 * half + 4.0 * (p // 4)
            lnanc[:, k] = np.log(ANCHOR_W[slab % A])
        out.append(np.concatenate([base, gzb, lnanc], axis=1).astype(np.float32))
    return out


def _run(inputs, trace=False):
    from concourse.bass_utils import run_bass_kernel_spmd

    x = np.ascontiguousarray(np.asarray(inputs["input"], dtype=np.float32))
    assert x.shape == (B, A * ATTRS, D, H, W), x.shape
    x12 = x.reshape(B * A, ATTRS, S)

    if "nc" not in _CACHE:
        _CACHE["nc"] = _build_nc()
        _CACHE["consts"] = _host_constants()
    nc = _CACHE["nc"]
    consts = _CACHE["consts"]

    in_maps = []
    for core in range(NCORES):
        parts = []
        for k in range(HS_PER_CORE):
            hs_g = HS_PER_CORE * core + k
            slab, half = divmod(hs_g, 2)
            parts.append(x12[slab, :, half * SH:(half + 1) * SH])
        in_maps.append({"xin": np.stack(parts), "consts": consts[core]})

    res = run_bass_kernel_spmd(
        nc, in_maps, core_ids=list(range(NCORES)), trace=trace
    )
    _CACHE["last_exec_ns"] = res.exec_time_ns
    _CACHE["last_results"] = res

    full = np.stack([res.results[c]["yout"] for c in range(NCORES)])
    return full.reshape(B, A * S, ATTRS)


def kernel(**inputs):
    return _run(inputs, trace=False)
